# revision 1
# baseline (speedup 1.0000x reference)
"""FlowNetC correlation (81-displacement cost volume) on 8 Trainium2 NeuronCores.

Problem: input1/input2 [8, 256, 64, 128] fp32 ->
         out[b, d=(i,j), h, w] = 1/256 * sum_c in1[b,c,h,w] * in2pad[b,c,h+i,w+j]
         (i, j in 0..8;  in2 zero-padded by 4 on each spatial side), out [8, 81, 64, 128].

Sharding: pure data-parallel, one batch element per core (SPMD, no collectives).

Host prep (cheap numpy, outside the hot loop): cast both inputs to fp16
(randn data; fp16 operands + fp32 PSUM accumulation => ~5e-4 rel err),
pre-scale in1 by 1/256, pre-block in1 pixels into contiguous 8x16 blocks,
pre-pad in2 spatially, and interleave the two 128-channel halves per tensor.

Per-core pipeline (all 64 pixel-blocks, software-pipelined over rings):
  PE     : 2 accumulating fp16 matmuls per block: lhsT = in1[c, 128 block
           pixels], rhs = 16r x 24s padded-in2 region -> PSUM Gram [128, 384]
           (col n = 24r + s); pixel p=(dh,dw) needs band n = f0(p) + 24i + j,
           f0(p) = 24 dh + dw.
  DVE    : evacuate PSUM -> SBUF fp16 (g16).
  ACT DMA: "skewed dump" g16 -> DRAM stage ring at addr p*385 + (f - f0(p)):
           per-partition windows land at a partition-UNIFORM pitch.  (SBUF APs
           cannot express per-partition offsets; DRAM-side strides can.)
  SP DMA : skew-read stage -> T2w[p, f'=24i+j] with plain [[385,128],[1,201]].
  ACT    : compact T2w window -> T2c[p, (i,j)] (strided copy).
  PE     : transpose T2c -> PSUM [81, 128] (displacements onto partitions).
  ACT    : copy into T4 [81, (dh, wb, dw)] staging.
  Pool   : per 8-row group, one fat SWDGE DMA T4 -> DRAM out (fp16->fp32 cast).
"""

from contextlib import ExitStack

import numpy as np

import concourse.bass as bass
import concourse.mybir as mybir
from concourse.ap import AP


C = 256
H = 64
W = 128
PAD = 4
HP = H + 2 * PAD  # 72
WP = W + 2 * PAD  # 136
D = 81
HB, WB = 8, 16  # pixel block: 8 rows x 16 cols = 128 partitions
NBH, NBW = H // HB, W // WB  # 8 x 8 = 64 blocks
NBLK = NBH * NBW
SRH, SRW = HB + 8, WB + 8  # rhs region 16r x 24s
NF = SRH * SRW  # 384 psum cols
NW = SRW * 8 + 9  # 201-element window: f' = 24i + j, i,j in 0..8
# Stage geometry (block-PAIR granularity): a pair of Gram blocks is dumped
# STRAIGHT (element (p, b2, f) at p*768 + b2*384 + f) with one trivial 2-dim
# DMA; each block's per-pixel window [f0(p), f0(p)+201) is then pulled back
# by a 3-dim flat-DRAM read [[12312, 8], [769, 16], [1, 201]] (the skew
# f0(p) = 24 dh + dw folds into the dh/dw strides).
SLOTP = 128 * 2 * NF  # straight pair slot (elements)
F16 = mybir.dt.float16
F32 = mybir.dt.float32

# ring depths (pairs where noted)
RG = 4  # psum gram ring (banks)
RT = 4  # psum transpose ring (banks)
RE = 12  # g16 slots (even: dump reads aligned slot pairs)
RSP = 8  # t2w pair ring
RCP = 8  # t2c pair ring
RDP = 10  # DRAM stage pair ring
RO = 2  # t4 ring

TL = 6  # PE: transpose(k) issued after matmuls of block k+TL (must clear the
# dump(m+CLP)->compact chain: needs mm through 2*CLP+1 blocks ahead)
CLP = 2  # ACT: compact-pair(m) issued at pair-iteration m+CLP
ELP = 2  # ACT: evac2 of pair m issued at pair-iteration m+ELP


def build_nc(reps: int = 1) -> bass.Bass:
    nc = bass.Bass("TRN2", target_bir_lowering=False)

    # host-prepped inputs: c-half interleaved on the free dim
    in1 = nc.dram_tensor("in1", [128, 2 * H * W], F16, kind="ExternalInput")
    in2 = nc.dram_tensor("in2", [128, 2 * HP * WP], F16, kind="ExternalInput")
    out = nc.dram_tensor("out", [D, H, W], F32, kind="ExternalOutput")
    stage = nc.dram_tensor("stage", [RDP, SLOTP], F16, kind="Internal")
    # tag input: makes each reps-variant's HLO unique so the neuronxcc cache
    # (keyed on HLO only) can't serve a stale NEFF across kernel revisions
    rtag = nc.dram_tensor("rtag", [1, reps], F16, kind="ExternalInput")

    with ExitStack() as ctx:
        ec = ctx.enter_context
        in1_sb = ec(nc.sbuf_tensor("in1_sb", [128, 2 * H * W], F16))
        p2_sb = ec(nc.sbuf_tensor("p2_sb", [128, 2 * HP * WP], F16))
        ident = ec(nc.sbuf_tensor("ident", [128, 128], F16))
        rtag_sb = ec(nc.sbuf_tensor("rtag_sb", [1, 16], F16))
        g16 = ec(nc.sbuf_tensor("g16", [128, RE * NF], F16))
        t2w = [ec(nc.sbuf_tensor(f"t2w_{i}", [128, 2 * NW], F16)) for i in range(RSP)]
        t2c = [ec(nc.sbuf_tensor(f"t2c_{i}", [128, 2 * D], F16)) for i in range(RCP)]
        t4 = [ec(nc.sbuf_tensor(f"t4_{i}", [D, NBW * HB * WB], F16)) for i in range(RO)]
        psum_g = [ec(nc.psum_tensor(f"psum_g{i}", [128, NF], F32)) for i in range(RG)]
        psum_t = [ec(nc.psum_tensor(f"psum_t{i}", [D, 256], F16)) for i in range(RT)]

        s_init = ec(nc.semaphore("s_init"))
        s_rtag = ec(nc.semaphore("s_rtag"))
        s_load_r = [ec(nc.semaphore(f"s_load{i}")) for i in range(NBH)]
        s_dump_r = [ec(nc.semaphore(f"s_dump{i}")) for i in range(RDP)]
        s_skew_r = [ec(nc.semaphore(f"s_skew{i}")) for i in range(RSP)]
        s_out_r = [ec(nc.semaphore(f"s_out{i}")) for i in range(RO)]
        s_mm = ec(nc.semaphore("s_mm"))
        s_e1 = ec(nc.semaphore("s_e1"))
        s_c = ec(nc.semaphore("s_c"))
        s_t = ec(nc.semaphore("s_t"))
        s_e2 = ec(nc.semaphore("s_e2"))

        def blk(k):
            return k // NBW, k % NBW  # (hb, wb)

        with nc.Block() as block:

            @block.gpsimd
            def _(gp):
                # identity for the PE transpose (ordered via s_init)
                ident_ap = AP(ident, 0, [[128, 128], [1, 128]])
                gp.memset(ident_ap, 0.0).then_inc(s_init, 1)
                gp.wait_ge(s_init, 1)
                gp.affine_select(
                    out=ident_ap, in_=ident_ap,
                    compare_op=mybir.AluOpType.not_equal, fill=1.0,
                    base=0, pattern=[[-1, 128]], channel_multiplier=1,
                ).then_inc(s_init, 1)
                gp.wait_ge(s_init, 2)
                gp.dma_start(
                    AP(rtag_sb, 0, [[16, 1], [1, reps]]),
                    AP(rtag, 0, [[reps, 1], [1, reps]]),
                ).then_inc(s_rtag, 16)
                # chunked input loads; round t covers in1 rows [8t, 8t+8) and
                # in2 pad-rows through 8t+16.
                for t in range(NBH):
                    gp.dma_start(
                        AP(in1_sb, t * HB * W, [[2 * H * W, 128], [H * W, 2], [1, HB * W]]),
                        AP(in1, t * HB * W, [[2 * H * W, 128], [H * W, 2], [1, HB * W]]),
                    ).then_inc(s_load_r[t], 16)
                    r0, r1 = (0, 2 * HB) if t == 0 else (HB * t + HB, HB * t + 2 * HB)
                    gp.dma_start(
                        AP(p2_sb, r0 * WP, [[2 * HP * WP, 128], [HP * WP, 2], [1, (r1 - r0) * WP]]),
                        AP(in2, r0 * WP, [[2 * HP * WP, 128], [HP * WP, 2], [1, (r1 - r0) * WP]]),
                    ).then_inc(s_load_r[t], 16)
                # output: one fat cast-DMA per 8-row group
                for hba in range(reps * NBH):
                    gp.wait_ge(s_e2, (hba + 1) * NBW // 2)
                    gp.dma_start(
                        AP(out, (hba % NBH) * HB * W, [[H * W, D], [1, NBW * HB * WB]]),
                        AP(t4[hba % RO], 0, [[NBW * HB * WB, D], [1, NBW * HB * WB]]),
                    ).then_inc(s_out_r[hba % RO], 16)
                gp.wait_ge(s_rtag, 16)

            @block.tensor
            def _(te):
                def do_transpose(k):
                    te.wait_ge(s_c, k // 2 + 1)
                    # (k is absolute across reps)
                    if k // 2 >= RT:
                        te.wait_ge(s_e2, k // 2 - RT + 1)
                    te.transpose(
                        AP(psum_t[(k // 2) % RT], (k % 2) * 128, [[256, D], [1, 128]]),
                        AP(t2c[(k // 2) % RCP], (k % 2) * D, [[2 * D, 128], [1, D]]),
                        AP(ident, 0, [[128, 128], [1, 128]]),
                    ).then_inc(s_t, 1)

                for k in range(reps * NBLK):
                    hb, wb = blk(k % NBLK)
                    if k < NBLK and wb == 0:
                        te.wait_ge(s_load_r[hb], 32)
                        if hb == 0:
                            te.wait_ge(s_init, 2)
                    if k >= RG:
                        te.wait_ge(s_e1, k - RG + 1)
                    h0, w0 = hb * HB, wb * WB
                    for hf in range(2):
                        mm = te.matmul(
                            AP(psum_g[k % RG], 0, [[NF, 128], [1, NF]]),
                            AP(in1_sb, hf * H * W + (k % NBLK) * 128, [[2 * H * W, 128], [1, 128]]),
                            AP(p2_sb, hf * HP * WP + h0 * WP + w0,
                               [[2 * HP * WP, 128], [WP, SRH], [1, SRW]]),
                            start=(hf == 0), stop=(hf == 1),
                        )
                    mm.then_inc(s_mm, 1)
                    if k >= TL:
                        do_transpose(k - TL)
                for k in range(reps * NBLK - TL, reps * NBLK):
                    do_transpose(k)

            @block.vector
            def _(ve):
                for k in range(reps * NBLK):
                    ve.wait_ge(s_mm, k + 1)
                    if k >= RE:
                        mp = (k - RE) // 2  # pair that last dumped this slot
                        ve.wait_ge(s_dump_r[mp % RDP], 16 * (mp // RDP + 1))
                    ve.tensor_copy(
                        AP(g16, (k % RE) * NF, [[RE * NF, 128], [1, NF]]),
                        AP(psum_g[k % RG], 0, [[NF, 128], [1, NF]]),
                    ).then_inc(s_e1, 1)

            @block.sync
            def _(sy):
                # per-block skewed window read out of the straight pair dump:
                # addr(p=(dh,dw), f') = p*768 + b2*384 + (24 dh + dw) + f'
                for m in range(reps * NBLK // 2):
                    sy.wait_ge(s_dump_r[m % RDP], 16 * (m // RDP + 1))
                    if m >= RSP:
                        sy.wait_ge(s_c, m - RSP + 1)
                    for b2 in range(2):
                        sy.dma_start(
                            AP(t2w[m % RSP], b2 * NW, [[2 * NW, 128], [1, NW]]),
                            AP(stage, (m % RDP) * SLOTP + b2 * NF,
                               [[16 * 2 * NF + SRW, 8], [2 * NF + 1, 16], [1, NW]]),
                        ).then_inc(s_skew_r[m % RSP], 16)

            @block.scalar
            def _(sc):
                def do_dump(m):
                    # straight pair dump: g16 slots (2m, 2m+1)%RE -> stage
                    sc.wait_ge(s_e1, 2 * m + 2)
                    if m >= RDP:
                        j = m - RDP  # stage pair-slot previously read by skew(j)
                        sc.wait_ge(s_skew_r[j % RSP], 32 * (j // RSP + 1))
                    slot = (2 * m) % RE
                    sc.dma_start(
                        AP(stage, (m % RDP) * SLOTP, [[2 * NF, 128], [1, 2 * NF]]),
                        AP(g16, slot * NF, [[RE * NF, 128], [1, 2 * NF]]),
                    ).then_inc(s_dump_r[m % RDP], 16)

                def do_compact(m):
                    sc.wait_ge(s_skew_r[m % RSP], 32 * (m // RSP + 1))
                    if m >= RCP:
                        sc.wait_ge(s_t, 2 * (m - RCP) + 2)
                    sc.copy(
                        AP(t2c[m % RCP], 0, [[2 * D, 128], [D, 2], [9, 9], [1, 9]]),
                        AP(t2w[m % RSP], 0, [[2 * NW, 128], [NW, 2], [SRW, 9], [1, 9]]),
                    ).then_inc(s_c, 1)

                def do_evac2(m):
                    # both blocks of pair m: wb slots (2m)%NBW, +1 of row hb
                    hb, wb = blk((2 * m) % NBLK)
                    hba = 2 * m // NBW  # absolute row index across reps
                    sc.wait_ge(s_t, 2 * m + 2)
                    if wb == 0 and hba >= RO:
                        sc.wait_ge(s_out_r[hba % RO], 16 * (hba // RO))
                    sc.copy(
                        AP(t4[hb % RO], wb * WB,
                           [[NBW * HB * WB, D], [WB, 2], [NBW * WB, HB], [1, WB]]),
                        AP(psum_t[m % RT], 0, [[256, D], [128, 2], [WB, HB], [1, WB]]),
                    ).then_inc(s_e2, 1)

                for m in range(reps * NBLK // 2):
                    do_dump(m)
                    if m >= CLP:
                        do_compact(m - CLP)
                    if m >= ELP:
                        do_evac2(m - ELP)
                for m in range(reps * NBLK // 2 - CLP, reps * NBLK // 2):
                    do_compact(m)
                for m in range(reps * NBLK // 2 - ELP, reps * NBLK // 2):
                    do_evac2(m)

    return nc


_nc_cache: list = []


def _get_nc() -> bass.Bass:
    if not _nc_cache:
        _nc_cache.append(build_nc())
    return _nc_cache[0]


def host_prep(input1: np.ndarray, input2: np.ndarray):
    """fp16 cast + 1/256 prescale (in1) + pixel-blocking (in1) + spatial pad
    (in2), both reshaped to [128, 2*...] with the c-half on the free dim."""
    B = input1.shape[0]
    i1 = (input1.astype(np.float32) * (1.0 / C)).astype(np.float16)
    i1 = (
        i1.reshape(B, 2, 128, NBH, HB, NBW, WB)
        .transpose(0, 2, 1, 3, 5, 4, 6)  # -> [B, 128, 2, hb, wb, dh, dw]
        .reshape(B, 128, 2 * H * W)
    )
    i2 = np.zeros((B, 2, 128, HP, WP), np.float16)
    i2[:, :, :, PAD : PAD + H, PAD : PAD + W] = (
        input2.astype(np.float16).reshape(B, 2, 128, H, W)
    )
    i2 = i2.transpose(0, 2, 1, 3, 4).reshape(B, 128, 2 * HP * WP)
    return np.ascontiguousarray(i1), np.ascontiguousarray(i2)


def kernel(input1: np.ndarray, input2: np.ndarray, *, trace: bool = False):
    """Full inputs [8, 256, 64, 128] fp32 -> full output [8, 81, 64, 128] fp32."""
    from concourse.bass_utils import run_bass_kernel_spmd

    input1 = np.asarray(input1)
    input2 = np.asarray(input2)
    B = input1.shape[0]
    i1, i2 = host_prep(input1, input2)
    nc = _get_nc()
    rt = np.zeros((1, 1), np.float16)
    in_maps = [{"in1": i1[b], "in2": i2[b], "rtag": rt} for b in range(B)]
    res = run_bass_kernel_spmd(nc, in_maps, core_ids=list(range(B)), trace=trace)
    out = np.stack([r["out"] for r in res.results]).astype(np.float32)
    if trace:
        kernel.last_results = res
    return out



# revision 2
# speedup vs baseline: 59.9603x; 59.9603x over previous
"""FlowNetC correlation (81-displacement cost volume) on 8 Trainium2 NeuronCores, v2.2.

out[b, (i,j), h, w] = 1/256 * sum_c in1[b,c,h,w] * in2pad[b,c,h+i,w+j]
in1/in2 [8, 256, 64, 128] f32; out [8, 81, 64, 128] f32; i,j in 0..8, pad 4.

Sharding: one batch element per core (SPMD, no collectives).

Host prep: fp16 cast + c-half interleave for both inputs; in1 pixel-blocked
(8x16 blocks) so the matmul lhsT is a 2-dim AP.  No padding (device memset)
and no prescale (folded into the evac).

Per-core pipeline (block = 8x16 pixels, 64 blocks; pair = 2; quad = 4):
  PE  : per block 2 accumulating f16 matmuls -> f32 PSUM Gram [128 pix, 384];
        per block 1 transpose t2c [128, 81] -> pst [81, 128] f16.
  DVE : evac pair (5 of 8): scalar_mul(1/256) f32->f16 PSUM -> g16 quad ring;
        compact quad (even quads): t2w window -> t2c 81-col picks.
  ACT : evac pair (3 of 8); compact (odd quads); dump quad: g16 -> DRAM stage
        (3KB/partition contiguous); evac2 per row-group: pst [81, 1024] -> t4.
  SP  : span-skew per quad: stage slot row [f0(p), f0(p)+1353) -> t2w (ONE
        2706B run per partition covers 4 blocks' windows; DRAM-side strides
        absorb f0(p) = 24*dh + dw); out per row-group: t4 [81, 1024] f16 ->
        out DRAM (f32 cast on host).
"""

from contextlib import ExitStack

import numpy as np

import concourse.bass as bass
import concourse.mybir as mybir
from concourse.ap import AP

C = 256
H = 64
W = 128
PAD = 4
HP = H + 2 * PAD  # 72
WP = W + 2 * PAD  # 136
D = 81
HB, WB = 8, 16
NBH, NBW = H // HB, W // WB  # 8 x 8
NBLK = NBH * NBW  # 64
SRH, SRW = HB + 8, WB + 8  # 16 x 24 rhs region
NF = SRH * SRW  # 384
NW = SRW * 8 + 9  # 201
SPAN = 3 * NF + NW  # 1353
F16 = mybir.dt.float16
F32 = mybir.dt.float32

NPAIR = NBLK // 2  # 32
NQUAD = NBLK // 4  # 16
RG = 3  # psum_g ring (pairs; 2 banks each = 6 banks)
REQ = 8  # g16 ring (quad slots)
RSTG = 8  # DRAM stage ring (quad slots)
RT2W = 8  # t2w ring (quad slots)
RTC = 8  # t2c ring (quad slots)
TSLOT = 1360  # t2w quad slot pitch (elements)
RO = 2  # t4 / pst ring (row slots)
TL = 24  # transpose lag (blocks)
CLAG = 3  # compact lag (quads)
SCALE = 1.0 / 256.0
QSLOT = 128 * 4 * NF  # stage quad slot (elements)
EV2Q = 8  # evac2(ra) issued at quad 2*ra + EV2Q
OUTQ = 9  # out(ra) issued at quad 2*ra + OUTQ


def _eo(p2: int) -> str:
    """Evac pair ownership: 5 of 8 on DVE, 3 of 8 on ACT."""
    return "dve" if p2 % 8 < 5 else "act"


def _eidx(p2: int) -> int:
    """Index of pair p2 among its owner's evacs."""
    if _eo(p2) == "dve":
        return 5 * (p2 // 8) + p2 % 8
    return 3 * (p2 // 8) + (p2 % 8 - 5)


def _co(qa: int) -> str:
    """Compact quad ownership: even on DVE, odd on ACT."""
    return "dve" if qa % 2 == 0 else "act"


class _Sems:
    """One full set of pipeline semaphores (the looped build has two)."""

    def __init__(self, ec, nc, tag=""):
        self.mm = ec(nc.semaphore(f"s_mm{tag}"))
        self.e1d = ec(nc.semaphore(f"s_e1d{tag}"))
        self.e1a = ec(nc.semaphore(f"s_e1a{tag}"))
        self.cd = ec(nc.semaphore(f"s_cd{tag}"))
        self.ca = ec(nc.semaphore(f"s_ca{tag}"))
        self.dump = [ec(nc.semaphore(f"s_dump{tag}{i}")) for i in range(RSTG)]
        self.skew = [ec(nc.semaphore(f"s_skew{tag}{i}")) for i in range(RT2W)]
        self.t = ec(nc.semaphore(f"s_t{tag}"))
        self.e2 = ec(nc.semaphore(f"s_e2{tag}"))
        self.outr = [ec(nc.semaphore(f"s_out{tag}{i}")) for i in range(RO)]

    def finals(self):
        f = []
        if STAGES >= 1:
            f.append((self.mm, NBLK))
        if STAGES >= 2:
            f += [(self.e1d, 20), (self.e1a, 12)]
        if STAGES >= 5:
            f += [(self.cd, 8), (self.ca, 8)]
        if STAGES >= 6:
            f.append((self.t, NBLK))
        if STAGES >= 7:
            f.append((self.e2, NBH))
        if STAGES >= 3:
            f += [(s, 16 * (NQUAD // RSTG)) for s in self.dump]
        if STAGES >= 4:
            f += [(s, 16 * (NQUAD // RT2W)) for s in self.skew]
        if STAGES >= 8:
            f += [(s, 16 * (NBH // RO)) for s in self.outr]
        return f


class _Bufs:
    def __init__(self, d):
        self.__dict__.update(d)


STAGES = 8  # build-time stage gate for the looped bisection variants


def _emit_pe(te, S, B, r0, r1, ident):
    """PE work for rep-range [r0, r1) with sems S, buffers B (index base 0)."""
    def do_transpose(ka):
        qa = ka // 4
        te.wait_ge(S.cd if _co(qa) == "dve" else S.ca, qa // 2 + 1)
        ra = ka // 8
        if STAGES >= 7 and ra >= RO:
            te.wait_ge(S.e2, ra - RO + 1)
        te.transpose(
            AP(B.pst[ra % RO], (ka % 8) * 128, [[1024, D], [1, 128]]),
            AP(B.t2c, B.tc_off + (qa % RTC) * 4 * D + (ka % 4) * D,
               [[B.tc_pitch, 128], [1, D]]),
            AP(ident, 0, [[128, 128], [1, 128]]),
        ).then_inc(S.t, 1)

    n = (r1 - r0) * NBLK
    if STAGES < 1:
        return
    for i in range(n):
        ka = i  # local index; rings/sems all local to this set
        p2a = ka // 2
        if STAGES >= 2 and ka % 2 == 0 and p2a >= RG:
            p = p2a - RG
            te.wait_ge(S.e1d if _eo(p) == "dve" else S.e1a, _eidx(p) + 1)
        k = ka % NBLK
        for hf in range(2):
            mm = te.matmul(
                AP(B.psum_g[p2a % RG], (ka % 2) * 512, [[1024, 128], [1, NF]]),
                AP(B.in1_sb, hf * H * W + k * 128, [[2 * H * W, 128], [1, 128]]),
                AP(B.p2_sb, hf * HP * WP + (k // NBW) * HB * WP + (k % NBW) * WB,
                   [[2 * HP * WP, 128], [WP, SRH], [1, SRW]]),
                start=(hf == 0), stop=(hf == 1),
            )
        mm.then_inc(S.mm, 1)
        if STAGES >= 6 and ka >= TL:
            do_transpose(ka - TL)
    if STAGES >= 6:
        for ka in range(n - TL, n):
            do_transpose(ka)


def _do_evac(eng, S, B, p2a):
    eng.wait_ge(S.mm, 2 * p2a + 2)
    qa = p2a // 2
    if STAGES >= 3 and qa >= REQ:
        j = qa - REQ
        eng.wait_ge(S.dump[j % RSTG], 16 * (j // RSTG + 1))
    dst = AP(B.g16, B.g_off + (qa % REQ) * 4 * NF + (p2a % 2) * 2 * NF,
             [[B.g_pitch, 128], [1, 2 * NF]])
    src = AP(B.psum_g[p2a % RG], 0, [[1024, 128], [512, 2], [1, NF]])
    sem = S.e1d if _eo(p2a) == "dve" else S.e1a
    if hasattr(eng, "tensor_scalar_mul"):
        eng.tensor_scalar_mul(dst, src, SCALE).then_inc(sem, 1)
    else:
        eng.mul(dst, src, SCALE).then_inc(sem, 1)


def _do_compact(eng, S, B, qa):
    eng.wait_ge(S.skew[qa % RT2W], 16 * (qa // RT2W + 1))
    if STAGES >= 6 and qa >= RTC:
        eng.wait_ge(S.t, 4 * (qa - RTC) + 4)
    cp = eng.tensor_copy if hasattr(eng, "tensor_copy") else eng.copy
    cp(
        AP(B.t2c, B.tc_off + (qa % RTC) * 4 * D,
           [[B.tc_pitch, 128], [D, 4], [9, 9], [1, 9]]),
        AP(B.t2w, B.w_off + (qa % RT2W) * TSLOT,
           [[B.w_pitch, 128], [NF, 4], [SRW, 9], [1, 9]]),
    ).then_inc(S.cd if _co(qa) == "dve" else S.ca, 1)


def _emit_dve(ve, S, B, r0, r1):
    if STAGES < 2:
        return
    for qa in range((r1 - r0) * NQUAD):
        for p2a in (2 * qa, 2 * qa + 1):
            if _eo(p2a) == "dve":
                _do_evac(ve, S, B, p2a)
        if STAGES >= 5 and qa >= CLAG and _co(qa - CLAG) == "dve":
            _do_compact(ve, S, B, qa - CLAG)
    n = (r1 - r0) * NQUAD
    if STAGES >= 5:
        for qa in range(n - CLAG, n):
            if _co(qa) == "dve":
                _do_compact(ve, S, B, qa)


def _do_dump(eng, S, B, qa):
    if STAGES < 3:
        return
    for p2a in (2 * qa, 2 * qa + 1):
        kind = "d" if _eo(p2a) == "dve" else "a"
        eng.wait_ge(S.e1d if kind == "d" else S.e1a, _eidx(p2a) + 1)
    if STAGES >= 4 and qa >= RSTG:
        j = qa - RSTG  # stage slot last read by skew of quad j
        eng.wait_ge(S.skew[j % RT2W], 16 * (j // RT2W + 1))
    eng.dma_start(
        AP(B.stage, B.st_off + (qa % RSTG) * QSLOT, [[4 * NF, 128], [1, 4 * NF]]),
        AP(B.g16, B.g_off + (qa % REQ) * 4 * NF, [[B.g_pitch, 128], [1, 4 * NF]]),
    ).then_inc(S.dump[qa % RSTG], 16)


def _do_skew(eng, S, B, qa):
    if STAGES < 4:
        return
    eng.wait_ge(S.dump[qa % RSTG], 16 * (qa // RSTG + 1))
    if STAGES >= 5 and qa >= RT2W:
        j = qa - RT2W  # t2w slot last read by compact of quad j
        eng.wait_ge(S.cd if _co(j) == "dve" else S.ca, j // 2 + 1)
    eng.dma_start(
        AP(B.t2w, B.w_off + (qa % RT2W) * TSLOT, [[B.w_pitch, 128], [1, SPAN]]),
        AP(B.stage, B.st_off + (qa % RSTG) * QSLOT,
           [[16 * 4 * NF + SRW, 8], [4 * NF + 1, 16], [1, SPAN]]),
    ).then_inc(S.skew[qa % RT2W], 16)


def _do_out(eng, S, B, ra, out):
    if STAGES < 8:
        return
    eng.wait_ge(S.e2, ra + 1)
    eng.dma_start(
        AP(out, (ra % NBH) * HB * W, [[H * W, D], [1, HB * W]]),
        AP(B.t4, B.t4_off + (ra % RO) * HB * W, [[B.t4_pitch, D], [1, HB * W]]),
    ).then_inc(S.outr[ra % RO], 16)


def _emit_act(sc, S, B, r0, r1, out):
    """ACT: its evacs, odd-quad compacts, odd-quad dumps, even-quad skews,
    evac2 per row, odd-row outs."""
    def do_evac2(ra):
        sc.wait_ge(S.t, 8 * ra + 8)
        if STAGES >= 8 and ra >= RO:
            sc.wait_ge(S.outr[ra % RO], 16 * (ra // RO))
        sc.copy(
            AP(B.t4, B.t4_off + (ra % RO) * HB * W,
               [[B.t4_pitch, D], [W, HB], [WB, NBW], [1, WB]]),
            AP(B.pst[ra % RO], 0, [[1024, D], [WB, HB], [128, NBW], [1, WB]]),
        ).then_inc(S.e2, 1)

    n = (r1 - r0) * NQUAD
    if STAGES < 2:
        return
    for qa in range(n):
        for p2a in (2 * qa, 2 * qa + 1):
            if _eo(p2a) == "act":
                _do_evac(sc, S, B, p2a)
        if STAGES >= 5 and qa >= CLAG and _co(qa - CLAG) == "act":
            _do_compact(sc, S, B, qa - CLAG)
        if qa % 2 == 1:
            _do_dump(sc, S, B, qa)
            _do_skew(sc, S, B, qa)
        if STAGES >= 7 and qa >= EV2Q and (qa - EV2Q) % 2 == 0:
            do_evac2((qa - EV2Q) // 2)
        if qa >= OUTQ and (qa - OUTQ) % 2 == 0 and ((qa - OUTQ) // 2) % 2 == 1:
            _do_out(sc, S, B, (qa - OUTQ) // 2, out)
    if STAGES >= 5:
        for qa in range(n - CLAG, n):
            if _co(qa) == "act":
                _do_compact(sc, S, B, qa)
    # tail: interleave evac2 with ACT's own (odd-row) outs so evac2(ra+2)'s
    # t4-reuse wait can see out(ra) already issued
    for ra in range(max(0, (r1 - r0) * NBH - EV2Q // 2), (r1 - r0) * NBH):
        if STAGES >= 7:
            do_evac2(ra)
        if ra % 2 == 1 and ra >= (r1 - r0) * NBH - (OUTQ - 1) // 2:
            _do_out(sc, S, B, ra, out)


def _emit_sp(sy, S, B, r0, r1, out):
    """SP: even-quad dumps, odd-quad skews, even-row outs."""
    n = (r1 - r0) * NQUAD
    for qa in range(n):
        if qa % 2 == 0:
            _do_dump(sy, S, B, qa)
            _do_skew(sy, S, B, qa)
        if qa >= OUTQ and (qa - OUTQ) % 2 == 0 and ((qa - OUTQ) // 2) % 2 == 0:
            _do_out(sy, S, B, (qa - OUTQ) // 2, out)
    for ra in range(max(0, (r1 - r0) * NBH - (OUTQ - 1) // 2), (r1 - r0) * NBH):
        if ra % 2 == 0:
            _do_out(sy, S, B, ra, out)


def _alloc(nc, ec, halves=1):
    d = {}
    d["in1_sb"] = ec(nc.sbuf_tensor("in1_sb", [128, 2 * H * W], F16))
    d["p2_sb"] = ec(nc.sbuf_tensor("p2_sb", [128, 2 * HP * WP], F16))
    d["g16"] = ec(nc.sbuf_tensor("g16", [128, halves * REQ * 4 * NF], F16))
    d["t2w"] = ec(nc.sbuf_tensor("t2w", [128, halves * RT2W * TSLOT], F16))
    d["t2c"] = ec(nc.sbuf_tensor("t2c", [128, halves * RTC * 4 * D], F16))
    d["t4"] = ec(nc.sbuf_tensor("t4", [D, halves * RO * HB * W], F16))
    d["psum_g"] = [ec(nc.psum_tensor(f"psum_g{i}", [128, 1024], F32)) for i in range(RG)]
    d["pst"] = [ec(nc.psum_tensor(f"pst{i}", [D, 1024], F16)) for i in range(RO)]
    d["g_pitch"] = halves * REQ * 4 * NF
    d["w_pitch"] = halves * RT2W * TSLOT
    d["tc_pitch"] = halves * RTC * 4 * D
    d["t4_pitch"] = halves * RO * HB * W
    return d


def _bufs_for_half(d, h, stage):
    return _Bufs(dict(
        d,
        stage=stage,
        g_off=h * REQ * 4 * NF,
        w_off=h * RT2W * TSLOT,
        tc_off=h * RTC * 4 * D,
        t4_off=h * RO * HB * W,
        st_off=h * RSTG * QSLOT,
    ))


def _preamble_gpsimd(gp, ident, p2_sb, s_init):
    ident_ap = AP(ident, 0, [[128, 128], [1, 128]])
    gp.memset(ident_ap, 0.0).then_inc(s_init, 1)
    gp.memset(AP(p2_sb, 0, [[2 * HP * WP, 128], [1, 2 * HP * WP]]), 0.0).then_inc(s_init, 1)
    gp.wait_ge(s_init, 2)
    gp.affine_select(
        out=ident_ap, in_=ident_ap,
        compare_op=mybir.AluOpType.not_equal, fill=1.0,
        base=0, pattern=[[-1, 128]], channel_multiplier=1,
    ).then_inc(s_init, 1)


def _load_in2(sy, p2_sb, in2, s_init, s_load):
    sy.wait_ge(s_init, 2)
    for hf in range(2):
        sy.dma_start(
            AP(p2_sb, hf * HP * WP + PAD * WP + PAD, [[2 * HP * WP, 128], [WP, H], [1, W]]),
            AP(in2, hf * H * W, [[2 * H * W, 128], [W, H], [1, W]]),
        ).then_inc(s_load, 16)


def build_nc(reps: int = 1) -> bass.Bass:
    global STAGES
    STAGES = 8
    nc = bass.Bass("TRN2", target_bir_lowering=False)

    in1 = nc.dram_tensor("in1", [128, 2 * H * W], F16, kind="ExternalInput")
    in2 = nc.dram_tensor("in2", [128, 2 * H * W], F16, kind="ExternalInput")
    out = nc.dram_tensor("out", [D, H * W], F16, kind="ExternalOutput")
    stage = nc.dram_tensor("stage", [RSTG, QSLOT], F16, kind="Internal")
    rtag = nc.dram_tensor("rtag", [1, reps], F16, kind="ExternalInput")

    with ExitStack() as ctx:
        ec = ctx.enter_context
        d = _alloc(nc, ec, halves=1)
        ident = ec(nc.sbuf_tensor("ident", [128, 128], F16))
        rtag_sb = ec(nc.sbuf_tensor("rtag_sb", [1, 16], F16))
        s_init = ec(nc.semaphore("s_init"))
        s_rtag = ec(nc.semaphore("s_rtag"))
        s_load = ec(nc.semaphore("s_load"))
        S = _Sems(ec, nc)
        B = _bufs_for_half(d, 0, stage)

        with nc.Block() as block:

            @block.gpsimd
            def _(gp):
                _preamble_gpsimd(gp, ident, d["p2_sb"], s_init)
                gp.dma_start(
                    AP(rtag_sb, 0, [[16, 1], [1, reps]]),
                    AP(rtag, 0, [[reps, 1], [1, reps]]),
                ).then_inc(s_rtag, 16)
                gp.wait_ge(s_rtag, 16)

            @block.sync
            def _(sy):
                _load_in2(sy, d["p2_sb"], in2, s_init, s_load)
                _emit_sp(sy, S, B, 0, reps, out)

            @block.tensor
            def _(te):
                te.wait_ge(s_load, 48)
                te.wait_ge(s_init, 3)
                _emit_pe(te, S, B, 0, reps, ident)

            @block.vector
            def _(ve):
                _emit_dve(ve, S, B, 0, reps)

            @block.scalar
            def _(sc):
                sc.dma_start(
                    AP(d["in1_sb"], 0, [[2 * H * W, 128], [1, 2 * H * W]]),
                    AP(in1, 0, [[2 * H * W, 128], [1, 2 * H * W]]),
                ).then_inc(s_load, 16)
                _emit_act(sc, S, B, 0, reps, out)

    return nc


def build_nc_loop(stages: int = 8) -> bass.Bass:
    """Hardware-looped timing variant: per-engine Fori loops over iterations of
    TWO reps each (A/B semaphore sets); SP is the barrier master.
    stages gates pipeline suffixes for HW cost bisection (8 = full)."""
    global STAGES
    STAGES = stages
    nc = bass.Bass("TRN2", target_bir_lowering=False, detect_race_conditions=False)
    I32 = mybir.dt.int32

    in1 = nc.dram_tensor("in1", [128, 2 * H * W], F16, kind="ExternalInput")
    in2 = nc.dram_tensor("in2", [128, 2 * H * W], F16, kind="ExternalInput")
    out = nc.dram_tensor("out", [D, H * W], F16, kind="ExternalOutput")
    stage = nc.dram_tensor("stage", [RSTG, QSLOT], F16, kind="Internal")
    lreps = nc.dram_tensor("lreps", [1, 2], I32, kind="ExternalInput")

    with ExitStack() as ctx:
        ec = ctx.enter_context
        d = _alloc(nc, ec, halves=1)
        ident = ec(nc.sbuf_tensor("ident", [128, 128], F16))
        lreps_sb = ec(nc.sbuf_tensor("lreps_sb", [1, 2 + 2 * stages], I32))
        s_init = ec(nc.semaphore("s_init"))
        s_load = ec(nc.semaphore("s_load"))
        X = [_Sems(ec, nc, "A"), _Sems(ec, nc, "B")]
        bar = [ec(nc.semaphore("s_barA")), ec(nc.semaphore("s_barB"))]
        go = [ec(nc.semaphore("s_goA")), ec(nc.semaphore("s_goB"))]
        BH = [_bufs_for_half(d, 0, stage), _bufs_for_half(d, 0, stage)]

        def worker_loop(eng, emit):
            eng.wait_ge(s_load, 64)
            with eng.register("r_end") as r:
                eng.reg_load(r, AP(lreps_sb, 0, [[2 + 2 * stages, 1], [1, 1]]))
                with eng.Fori(0, r):
                    for h in range(2):
                        emit(h)
                        eng.sem_inc(bar[h], 1)
                        eng.wait_ge(go[h], 1)

        with nc.Block() as block:

            @block.gpsimd
            def _(gp):
                _preamble_gpsimd(gp, ident, d["p2_sb"], s_init)

            @block.tensor
            def _(te):
                te.wait_ge(s_init, 3)
                worker_loop(te, lambda h: _emit_pe(te, X[h], BH[h], 0, 1, ident))

            @block.vector
            def _(ve):
                worker_loop(ve, lambda h: _emit_dve(ve, X[h], BH[h], 0, 1))

            @block.scalar
            def _(sc):
                sc.dma_start(
                    AP(d["in1_sb"], 0, [[2 * H * W, 128], [1, 2 * H * W]]),
                    AP(in1, 0, [[2 * H * W, 128], [1, 2 * H * W]]),
                ).then_inc(s_load, 16)
                worker_loop(sc, lambda h: _emit_act(sc, X[h], BH[h], 0, 1, out))

            @block.sync
            def _(sy):
                _load_in2(sy, d["p2_sb"], in2, s_init, s_load)
                sy.dma_start(
                    AP(lreps_sb, 0, [[2 + 2 * stages, 1], [1, 2]]),
                    AP(lreps, 0, [[2, 1], [1, 2]]),
                ).then_inc(s_load, 16)
                sy.wait_ge(s_load, 64)
                # seed: as if a previous half-B completed
                sy.sem_inc(go[1], 1)
                with sy.register("r_end") as r:
                    sy.reg_load(r, AP(lreps_sb, 0, [[2 + 2 * stages, 1], [1, 1]]))
                    with sy.Fori(0, r):
                        for h in range(2):
                            S = X[h]
                            _emit_sp(sy, S, BH[h], 0, 1, out)
                            sy.wait_ge(bar[h], 3)
                            finals = S.finals() + [(bar[h], 3), (go[1 - h], 1)]
                            for sem, v in finals:
                                sy.wait_ge(sem, v)
                            for sem, v in finals:
                                sy.sem_clear(sem)
                            sy.sem_inc(go[h], 1)

    return nc


_nc_cache: list = []


def _get_nc() -> bass.Bass:
    if not _nc_cache:
        _nc_cache.append(build_nc())
    return _nc_cache[0]


def host_prep(input1: np.ndarray, input2: np.ndarray):
    """fp16 cast + c-half interleave; in1 additionally pixel-blocked (the
    matmul stationary operand allows only one free dim)."""
    B = input1.shape[0]
    i1 = np.asarray(input1).astype(np.float16)
    # [B, 2, c128, hb, dh, wb, dw] -> [B, c128, 2, hb, wb, dh, dw]
    i1 = (
        i1.reshape(B, 2, 128, NBH, HB, NBW, WB)
        .transpose(0, 2, 1, 3, 5, 4, 6)
        .reshape(B, 128, 2 * H * W)
    )
    i2 = np.asarray(input2).astype(np.float16)
    i2 = i2.reshape(B, 2, 128, H * W).transpose(0, 2, 1, 3).reshape(B, 128, 2 * H * W)
    return np.ascontiguousarray(i1), np.ascontiguousarray(i2)


def kernel(input1: np.ndarray, input2: np.ndarray, *, trace: bool = False):
    """Full inputs [8, 256, 64, 128] f32 -> full output [8, 81, 64, 128] f32."""
    from concourse.bass_utils import run_bass_kernel_spmd

    input1 = np.asarray(input1)
    input2 = np.asarray(input2)
    B = input1.shape[0]
    i1, i2 = host_prep(input1, input2)
    nc = _get_nc()
    rt = np.zeros((1, 1), np.float16)
    in_maps = [{"in1": i1[b], "in2": i2[b], "rtag": rt} for b in range(B)]
    res = run_bass_kernel_spmd(nc, in_maps, core_ids=list(range(B)), trace=trace)
    out = np.stack([r["out"] for r in res.results]).astype(np.float32)
    if trace:
        kernel.last_results = res
    return out.reshape(B, D, H, W)
